# revision 30
# baseline (speedup 1.0000x reference)
"""Causal single-head attention (B=8, S=2048, D=512) on 8 TRN2 NeuronCores.

Strategy: data-parallel over the batch dim — one batch element per core.
Reference math per batch element:
    Q = q @ Wq.T + bq ; K = k @ Wk.T + bk ; V = v @ Wv.T + bv
    scores = Q @ K.T / sqrt(D)  (causal) ; out = softmax(scores) @ V
Algebra used on device:
  - bk drops out exactly (softmax is invariant to per-row score shifts).
  - The K projection is never materialized: with N = Wq^T @ Wk and
    u = Wk^T bq (both pure weight products, precomputed on host),
        scores^T = k @ (q @ N + u)^T
    so one big projection H' = q @ N + u replaces the Q and K
    projections AND the bq bias: the +u per-H-element constant turns
    into the per-key additive constant c = k@u inside the score matmul.
  - softmax runs without max-subtraction: scores are O(+-6) here so
    fp32 exp() cannot overflow/underflow.
  - bv is folded into the V projection; with late normalization
    out = (P_unnorm @ V) * (1/rowsum) the bias passes through exactly
    because rowsum comes from the same unnormalized P.
Layout: q/k/v are host-pre-arranged so every DMA line is 4-8KB
contiguous per partition and the contraction dim lands on partitions.
Score tiles are computed transposed ([s_k=128, s_q<=512]) so the
exp'd P tiles feed the PV matmul directly as stationary operands.
Row sums come from a ones column appended to V, with the PV output
split 256+257 across two PSUM banks. Only lower-triangular 128-col
blocks are computed; the 16 diagonal sub-tiles are masked with a 0/1
triangle. Matmul operands are bf16; PSUM accumulation / softmax
normalization stay fp32; the DRAM output is bf16 (re-widened on host).
Startup: inputs stream on the two fast HWDGE queues (sync + scalar)
in strict need-order — streams serialize within a queue, so the first
H^T operands are not starved by later transfers. A dummy-matmul
warm-up releases the PE HAM clock throttle while they fly.
"""

import numpy as np

B, S, D, P = 8, 2048, 512, 128
DC = D // P  # d-chunks (4)
NQB = S // P  # 128-row q/k blocks (16)
QW = 512  # q window (score-tile free dim)
NQC = S // QW  # q-chunks (4)
N_CORES = 8
N_WARM = 8  # dummy warm-up matmuls (N=512, cold ~427ns each)

_CACHE = {}


def _build(causal=True):
    import concourse.tile as tile
    from concourse import bacc, mybir
    from contextlib import ExitStack

    F32 = mybir.dt.float32
    MDT = mybir.dt.bfloat16
    AF = mybir.ActivationFunctionType

    nc = bacc.Bacc("TRN2", target_bir_lowering=False, debug=False)

    NU = P + 4  # nt inner: 128 N cols + u (at col 128) + padding
    qT = nc.dram_tensor("qT", [P, NQC, DC, QW], MDT, kind="ExternalInput").ap()
    kT = nc.dram_tensor("kT", [P, NQB, DC, P], MDT, kind="ExternalInput").ap()
    vT = nc.dram_tensor("vT", [P, NQB, DC, P], MDT, kind="ExternalInput").ap()
    ntT = nc.dram_tensor("ntT", [P, DC, DC, NU], MDT, kind="ExternalInput").ap()
    wvT = nc.dram_tensor("wvT", [P, DC, D], MDT, kind="ExternalInput").ap()
    bvb = nc.dram_tensor("bvb", [P, D], MDT, kind="ExternalInput").ap()
    cm = nc.dram_tensor("cm", [P, P], MDT, kind="ExternalInput").ap()
    out_d = nc.dram_tensor("out", [S, D], MDT, kind="ExternalOutput").ap()

    with tile.TileContext(nc) as tc, ExitStack() as ctx:
        consts = ctx.enter_context(tc.tile_pool(name="consts", bufs=1))
        acts = ctx.enter_context(tc.tile_pool(name="acts", bufs=1))
        ptpool = ctx.enter_context(tc.tile_pool(name="ptpool", bufs=18))
        opool = ctx.enter_context(tc.tile_pool(name="opool", bufs=2))
        small = ctx.enter_context(tc.tile_pool(name="small", bufs=4))
        psmm = ctx.enter_context(tc.tile_pool(name="psmm", bufs=4, space="PSUM"))
        psout = ctx.enter_context(tc.tile_pool(name="psout", bufs=2, space="PSUM"))

        cmask = consts.tile([P, P], MDT)
        bias_vb = consts.tile([P, D], MDT)

        # persistent per-core activations / resident inputs
        ht_sb = acts.tile([P, DC, S], MDT, tag="ht")  # H'^T[d2, s]
        kin = acts.tile([P, NQB, DC, P], MDT, tag="kin")  # k^T (resident)
        v_sb = acts.tile([P, NQB, D + 1], MDT, tag="v")  # V[s, e] (+bv) | ones
        nt_sb = acts.tile([P, DC, DC, NU], MDT, tag="nt")  # N[d1,d2]|u, host-made
        qt_in = acts.tile([P, NQC, DC, QW], MDT, tag="qt")  # q^T input
        vt_in = acts.tile([P, NQB, DC, P], MDT, tag="vt")  # v^T input
        wv_sb = acts.tile([P, DC, D], MDT, tag="w")

        # ---- warm-up + DMAs ----
        # PE warm-up first: matmuls on bias_vb BEFORE its (late, gpsimd)
        # DMA — contents are garbage, results discarded; the WAR hazard
        # just orders that DMA after the warm-up. Releases the HAM clock
        # throttle while the input DMAs fly, with no memset dependency.
        wps = psmm.tile([P, QW], F32, tag="mm")
        for _ in range(N_WARM):
            nc.tensor.matmul(wps, bias_vb[:, :P], bias_vb, start=True, stop=True)

        nc.gpsimd.memset(v_sb[:, :, D : D + 1], 1.0)  # PV rowsum ones column

        # The two HWDGE queues stream inputs in strict need-order
        # (streams serialize within a queue, queues race each other).
        nc.sync.dma_start(out=qt_in[:, 0], in_=qT[:, 0])
        nc.sync.dma_start(out=qt_in[:, 1], in_=qT[:, 1])
        nc.sync.dma_start(out=vt_in[:, :8], in_=vT[:, :8])
        nc.sync.dma_start(out=kin[:, :8], in_=kT[:, :8])
        nc.sync.dma_start(out=kin[:, 8:], in_=kT[:, 8:])

        for dcm in range(DC):  # nt in dcm-quarters: first H^T chain ASAP
            nc.scalar.dma_start(out=nt_sb[:, dcm], in_=ntT[:, dcm])
        nc.scalar.dma_start(out=qt_in[:, 2], in_=qT[:, 2])
        nc.scalar.dma_start(out=qt_in[:, 3], in_=qT[:, 3])
        nc.scalar.dma_start(out=wv_sb, in_=wvT)
        nc.scalar.dma_start(out=vt_in[:, 8:], in_=vT[:, 8:])

        # gpsimd's queue is slow to start; only late-needed constants.
        nc.gpsimd.dma_start(out=cmask, in_=cm)
        nc.gpsimd.dma_start(out=bias_vb, in_=bvb)

        # ---- H'^T = N^T q^T + u  (single projection, u folded in) ----
        # Chain (sc, dcm) needs only nt quarter dcm, which streams in
        # dcm-order, so chunk 0's chains start as the quarters land.
        inv_sqrt_d = float(1.0 / np.sqrt(D))
        for sc in range(NQC):
            for dcm in range(DC):
                ps = psmm.tile([P, QW], F32, tag="mm")
                for dpc in range(DC):
                    nc.tensor.matmul(
                        ps,
                        nt_sb[:, dcm, dpc, 0:P],
                        qt_in[:, sc, dpc, :],
                        start=(dpc == 0),
                        stop=(dpc == DC - 1),
                    )
                nc.scalar.add(
                    ht_sb[:, dcm, sc * QW : (sc + 1) * QW], ps,
                    nt_sb[:, dcm, 0, P : P + 1],
                )

        # ---- V projection: out[s, e] = sum_d v[s, d] W[e, d] + bv ----
        for sb in range(NQB):
            ps = psmm.tile([P, QW], F32, tag="mm")
            for dc in range(DC):
                nc.tensor.matmul(
                    ps,
                    vt_in[:, sb, dc, :],
                    wv_sb[:, dc, :],
                    start=(dc == 0),
                    stop=(dc == DC - 1),
                )
            nc.vector.tensor_add(v_sb[:, sb, 0:D], ps, bias_vb)

        # ---- attention, per 512-wide q chunk ----
        for qc in range(NQC):
            nkb = 4 * qc + 4 if causal else NQB  # causal: k-blocks 0..4qc+3
            pts = []
            for kb in range(nkb):
                t = kb - 4 * qc if causal else -1  # >=0: diagonal group
                off = max(0, t) * P  # columns below the diagonal are never read
                ps = psmm.tile([P, QW], F32, tag="mm")
                for dc in range(DC):
                    nc.tensor.matmul(
                        ps[:, off:],
                        kin[:, kb, dc, :],
                        ht_sb[:, dc, qc * QW + off : (qc + 1) * QW],
                        start=(dc == 0),
                        stop=(dc == DC - 1),
                    )
                pt = ptpool.tile([P, QW], MDT, tag="pt")
                nc.scalar.activation(
                    pt[:, off:], ps[:, off:], AF.Exp, scale=inv_sqrt_d,
                )
                if t >= 0:  # diagonal block: mask its triangular 128x128 sub-tile
                    nc.vector.tensor_mul(
                        pt[:, off : off + P], pt[:, off : off + P], cmask
                    )
                pts.append(pt)
            og = opool.tile([P, 4, D], MDT, tag="ot")
            HB = D // 2  # split PV output across two PSUM banks:
            for j in range(4):  # bank0: cols 0:256, bank1: cols 256:512 + rowsum
                qb = 4 * qc + j
                po = psout.tile([P, 2, QW], F32, tag="po")
                kb_hi = qb if causal else NQB - 1
                for kb in range(kb_hi + 1):
                    lhsT = pts[kb][:, j * P : (j + 1) * P]
                    nc.tensor.matmul(
                        po[:, 0, 0:HB], lhsT, v_sb[:, kb, 0:HB],
                        start=(kb == 0), stop=(kb == kb_hi),
                    )
                    nc.tensor.matmul(
                        po[:, 1, 0 : HB + 1], lhsT, v_sb[:, kb, HB : D + 1],
                        start=(kb == 0), stop=(kb == kb_hi),
                    )
                rec = small.tile([P, 1], F32, tag="rec")
                nc.vector.reciprocal(rec, po[:, 1, HB : HB + 1])
                if qb == NQB - 1:
                    # final block: normalize + store in quarters pipelined
                    # across both DMA queues to shorten the kernel tail.
                    for qtr in range(4):
                        b = (qtr % 2) * P
                        nc.vector.tensor_scalar_mul(
                            og[:, j, qtr * P : (qtr + 1) * P],
                            po[:, qtr // 2, b : b + P], rec,
                        )
                        dq = nc.sync if qtr % 2 == 0 else nc.scalar
                        dq.dma_start(
                            out=out_d[qb * P : (qb + 1) * P, qtr * P : (qtr + 1) * P],
                            in_=og[:, j, qtr * P : (qtr + 1) * P],
                        )
                else:
                    nc.vector.tensor_scalar_mul(og[:, j, 0:HB], po[:, 0, 0:HB], rec)
                    nc.vector.tensor_scalar_mul(og[:, j, HB:D], po[:, 1, 0:HB], rec)
                    nc.sync.dma_start(
                        out=out_d[qb * P : (qb + 1) * P, :], in_=og[:, j, :]
                    )

    nc.compile()
    return nc


def _get_nc(causal=True):
    key = ("nc", causal)
    if key not in _CACHE:
        _CACHE[key] = _build(causal)
    return _CACHE[key]


def _make_in_maps(q, k, v, Wq, bq, Wk, Wv, bv):
    import ml_dtypes

    mdt = ml_dtypes.bfloat16
    q = np.asarray(q, dtype=np.float32)
    k = np.asarray(k, dtype=np.float32)
    v = np.asarray(v, dtype=np.float32)

    def xq(x):  # [s, d] -> [p, sc, dc, qw] with d = dc*P + p, s = sc*QW + qw
        xt = np.ascontiguousarray(x.T).reshape(DC, P, NQC, QW)
        return np.ascontiguousarray(xt.transpose(1, 2, 0, 3)).astype(mdt)

    def xkv(x):  # [s, d] -> [p, sb, dc, ss] with d = dc*P + p, s = sb*P + ss
        xt = np.ascontiguousarray(x.T).reshape(DC, P, NQB, P)
        return np.ascontiguousarray(xt.transpose(1, 2, 0, 3)).astype(mdt)

    # host-precomputed weight products: N = Wq^T Wk, u = Wk^T bq.
    # nt layout [p, dcm, dpc, 0:128] = N[dpc*P+p, dcm*P+m]; u at col 128
    # of each dcm's dpc=0 plane (bias for the ht store).
    NT = np.asarray(Wq, np.float32).T @ np.asarray(Wk, np.float32)  # [d1, d2]
    u = np.asarray(Wk, np.float32).T @ np.asarray(bq, np.float32)  # [d]
    nt_t = np.zeros((P, DC, DC, P + 4), np.float32)
    # NT[dpc*P+p, dcm*P+m] -> [p, dcm, dpc, m]
    nt_t[:, :, :, :P] = NT.reshape(DC, P, DC, P).transpose(1, 2, 0, 3)
    nt_t[:, :, 0, P] = u.reshape(DC, P).transpose(1, 0)
    nt_t = np.ascontiguousarray(nt_t).astype(mdt)
    wt = np.asarray(Wv, np.float32).T.reshape(DC, P, D)
    wv_t = np.ascontiguousarray(wt.transpose(1, 0, 2)).astype(mdt)
    bvb = np.ascontiguousarray(
        np.tile(np.asarray(bv, dtype=np.float32)[None, :], (P, 1))
    ).astype(mdt)
    cm = np.triu(np.ones((P, P), dtype=np.float32)).astype(mdt)  # cm[kk,qq]=qq>=kk
    in_maps = []
    for c in range(N_CORES):
        in_maps.append(
            {
                "qT": xq(q[c]),
                "kT": xkv(k[c]),
                "vT": xkv(v[c]),
                "ntT": nt_t,
                "wvT": wv_t,
                "bvb": bvb,
                "cm": cm,
            }
        )
    return in_maps


def _run(in_maps, trace=False, causal=True):
    from concourse.bass_utils import run_bass_kernel_spmd

    nc = _get_nc(causal)
    res = run_bass_kernel_spmd(
        nc, in_maps, core_ids=list(range(N_CORES)), trace=trace
    )
    out = np.stack(
        [np.asarray(res.results[c]["out"]).astype(np.float32) for c in range(N_CORES)],
        axis=0,
    )
    return out, res


def _mask_is_causal(mask):
    m = np.asarray(mask).reshape(S, S).astype(bool)
    if m.all():
        return False  # attend-to-everything mask: run the dense variant
    tril = np.tril(np.ones((S, S), dtype=bool))
    if np.array_equal(m, tril):
        return True
    raise ValueError("unsupported mask pattern (expected causal or all-ones)")


def kernel(q, k, v, mask, Wq, bq, Wk, bk, Wv, bv):
    q = np.asarray(q, dtype=np.float32)
    assert q.shape == (B, S, D), f"unexpected q shape {q.shape}"
    causal = _mask_is_causal(mask)
    in_maps = _make_in_maps(q, k, v, Wq, bq, Wk, Wv, bv)
    out, _ = _run(in_maps, trace=False, causal=causal)
    return out


# revision 32
# speedup vs baseline: 1.0477x; 1.0477x over previous
"""Causal single-head attention (B=8, S=2048, D=512) on 8 TRN2 NeuronCores.

Strategy: data-parallel over the batch dim — one batch element per core.
Reference math per batch element:
    Q = q @ Wq.T + bq ; K = k @ Wk.T + bk ; V = v @ Wv.T + bv
    scores = Q @ K.T / sqrt(D)  (causal) ; out = softmax(scores) @ V
Algebra used on device:
  - bk drops out exactly (softmax is invariant to per-row score shifts).
  - The K projection is never materialized: with N = Wq^T @ Wk and
    u = Wk^T bq (both pure weight products, precomputed on host),
        scores^T = k @ (q @ N + u)^T
    so one big projection H' = q @ N + u replaces the Q and K
    projections AND the bq bias: the +u per-H-element constant turns
    into the per-key additive constant c = k@u inside the score matmul.
  - softmax runs without max-subtraction: scores are O(+-6) here so
    fp32 exp() cannot overflow/underflow.
  - bv is folded into the V projection; with late normalization
    out = (P_unnorm @ V) * (1/rowsum) the bias passes through exactly
    because rowsum comes from the same unnormalized P.
Layout: q/k/v are host-pre-arranged so every DMA line is 4-8KB
contiguous per partition and the contraction dim lands on partitions.
Score tiles are computed transposed ([s_k=128, s_q<=512]) so the
exp'd P tiles feed the PV matmul directly as stationary operands.
Row sums come from a ones column appended to V, with the PV output
split 256+257 across two PSUM banks. Only lower-triangular 128-col
blocks are computed; the 16 diagonal sub-tiles are masked with a 0/1
triangle. Matmul operands are bf16; PSUM accumulation / softmax
normalization stay fp32; the DRAM output is bf16 (re-widened on host).
Startup: inputs stream on the two fast HWDGE queues (sync + scalar)
in strict need-order — streams serialize within a queue, so the first
H^T operands are not starved by later transfers. A dummy-matmul
warm-up releases the PE HAM clock throttle while they fly.
"""

import numpy as np

B, S, D, P = 8, 2048, 512, 128
DC = D // P  # d-chunks (4)
NQB = S // P  # 128-row q/k blocks (16)
QW = 512  # q window (score-tile free dim)
NQC = S // QW  # q-chunks (4)
N_CORES = 8
N_WARM = 13  # dummy warm-up matmuls (N=512, cold ~427ns each)

_CACHE = {}


def _build(causal=True):
    import concourse.tile as tile
    from concourse import bacc, mybir
    from contextlib import ExitStack

    F32 = mybir.dt.float32
    MDT = mybir.dt.bfloat16
    AF = mybir.ActivationFunctionType

    nc = bacc.Bacc("TRN2", target_bir_lowering=False, debug=False)

    DU = D + 2  # nt planes carry u in column D (col D+1 is padding)
    qT = nc.dram_tensor("qT", [P, NQC, DC, QW], MDT, kind="ExternalInput").ap()
    kT = nc.dram_tensor("kT", [P, NQB, DC, P], MDT, kind="ExternalInput").ap()
    vT = nc.dram_tensor("vT", [P, NQB, DC, P], MDT, kind="ExternalInput").ap()
    ntT = nc.dram_tensor("ntT", [P, DC, DU], MDT, kind="ExternalInput").ap()
    wvT = nc.dram_tensor("wvT", [P, DC, D], MDT, kind="ExternalInput").ap()
    bvb = nc.dram_tensor("bvb", [P, D], MDT, kind="ExternalInput").ap()
    cm = nc.dram_tensor("cm", [P, P], MDT, kind="ExternalInput").ap()
    out_d = nc.dram_tensor("out", [S, D], MDT, kind="ExternalOutput").ap()

    with tile.TileContext(nc) as tc, ExitStack() as ctx:
        consts = ctx.enter_context(tc.tile_pool(name="consts", bufs=1))
        acts = ctx.enter_context(tc.tile_pool(name="acts", bufs=1))
        ptpool = ctx.enter_context(tc.tile_pool(name="ptpool", bufs=18))
        opool = ctx.enter_context(tc.tile_pool(name="opool", bufs=2))
        small = ctx.enter_context(tc.tile_pool(name="small", bufs=4))
        psmm = ctx.enter_context(tc.tile_pool(name="psmm", bufs=4, space="PSUM"))
        psout = ctx.enter_context(tc.tile_pool(name="psout", bufs=2, space="PSUM"))

        cmask = consts.tile([P, P], MDT)
        bias_vb = consts.tile([P, D], MDT)

        # persistent per-core activations / resident inputs
        ht_sb = acts.tile([P, DC, S], MDT, tag="ht")  # H'^T[d2, s]
        kin = acts.tile([P, NQB, DC, P], MDT, tag="kin")  # k^T (resident)
        v_sb = acts.tile([P, NQB, D + 1], MDT, tag="v")  # V[s, e] (+bv) | ones
        nt_sb = acts.tile([P, DC, DU], MDT, tag="nt")  # N[d1, d2] | u, host-made
        qt_in = acts.tile([P, NQC, DC, QW], MDT, tag="qt")  # q^T input
        vt_in = acts.tile([P, NQB, DC, P], MDT, tag="vt")  # v^T input
        wv_sb = acts.tile([P, DC, D], MDT, tag="w")

        # ---- warm-up + DMAs ----
        # PE warm-up first: matmuls on bias_vb BEFORE its (late, gpsimd)
        # DMA — contents are garbage, results discarded; the WAR hazard
        # just orders that DMA after the warm-up. Releases the HAM clock
        # throttle while the input DMAs fly, with no memset dependency.
        wps = psmm.tile([P, QW], F32, tag="mm")
        for _ in range(N_WARM):
            nc.tensor.matmul(wps, bias_vb[:, :P], bias_vb, start=True, stop=True)

        nc.gpsimd.memset(v_sb[:, :, D : D + 1], 1.0)  # PV rowsum ones column

        # The two HWDGE queues stream inputs in strict need-order
        # (streams serialize within a queue, queues race each other).
        nc.sync.dma_start(out=qt_in[:, 0], in_=qT[:, 0])
        nc.sync.dma_start(out=qt_in[:, 1], in_=qT[:, 1])
        nc.sync.dma_start(out=vt_in[:, :8], in_=vT[:, :8])
        nc.sync.dma_start(out=kin[:, :8], in_=kT[:, :8])
        nc.sync.dma_start(out=kin[:, 8:], in_=kT[:, 8:])

        nc.scalar.dma_start(out=nt_sb, in_=ntT)
        nc.scalar.dma_start(out=qt_in[:, 2], in_=qT[:, 2])
        nc.scalar.dma_start(out=qt_in[:, 3], in_=qT[:, 3])
        nc.scalar.dma_start(out=wv_sb, in_=wvT)
        nc.scalar.dma_start(out=vt_in[:, 8:], in_=vT[:, 8:])

        # gpsimd's queue is slow to start; only late-needed constants.
        nc.gpsimd.dma_start(out=cmask, in_=cm)
        nc.gpsimd.dma_start(out=bias_vb, in_=bvb)

        # ---- H'^T = N^T q^T + u  (single projection, u folded in) ----
        # Chain (sc, dcm) needs only nt quarter dcm, which streams in
        # dcm-order, so chunk 0's chains start as the quarters land.
        inv_sqrt_d = float(1.0 / np.sqrt(D))
        for sc in range(NQC):
            for dcm in range(DC):
                ps = psmm.tile([P, QW], F32, tag="mm")
                for dpc in range(DC):
                    nc.tensor.matmul(
                        ps,
                        nt_sb[:, dpc, dcm * P : (dcm + 1) * P],
                        qt_in[:, sc, dpc, :],
                        start=(dpc == 0),
                        stop=(dpc == DC - 1),
                    )
                nc.scalar.add(
                    ht_sb[:, dcm, sc * QW : (sc + 1) * QW], ps,
                    nt_sb[:, dcm, D : D + 1],
                )

        # ---- V projection: out[s, e] = sum_d v[s, d] W[e, d] + bv ----
        for sb in range(NQB):
            ps = psmm.tile([P, QW], F32, tag="mm")
            for dc in range(DC):
                nc.tensor.matmul(
                    ps,
                    vt_in[:, sb, dc, :],
                    wv_sb[:, dc, :],
                    start=(dc == 0),
                    stop=(dc == DC - 1),
                )
            nc.vector.tensor_add(v_sb[:, sb, 0:D], ps, bias_vb)

        # ---- attention, per 512-wide q chunk ----
        for qc in range(NQC):
            nkb = 4 * qc + 4 if causal else NQB  # causal: k-blocks 0..4qc+3
            pts = []
            for kb in range(nkb):
                t = kb - 4 * qc if causal else -1  # >=0: diagonal group
                off = max(0, t) * P  # columns below the diagonal are never read
                ps = psmm.tile([P, QW], F32, tag="mm")
                for dc in range(DC):
                    nc.tensor.matmul(
                        ps[:, off:],
                        kin[:, kb, dc, :],
                        ht_sb[:, dc, qc * QW + off : (qc + 1) * QW],
                        start=(dc == 0),
                        stop=(dc == DC - 1),
                    )
                pt = ptpool.tile([P, QW], MDT, tag="pt")
                nc.scalar.activation(
                    pt[:, off:], ps[:, off:], AF.Exp, scale=inv_sqrt_d,
                )
                if t >= 0:  # diagonal block: mask its triangular 128x128 sub-tile
                    nc.vector.tensor_mul(
                        pt[:, off : off + P], pt[:, off : off + P], cmask
                    )
                pts.append(pt)
            og = opool.tile([P, 4, D], MDT, tag="ot")
            HB = D // 2  # split PV output across two PSUM banks:
            for j in range(4):  # bank0: cols 0:256, bank1: cols 256:512 + rowsum
                qb = 4 * qc + j
                po = psout.tile([P, 2, QW], F32, tag="po")
                kb_hi = qb if causal else NQB - 1
                for kb in range(kb_hi + 1):
                    lhsT = pts[kb][:, j * P : (j + 1) * P]
                    nc.tensor.matmul(
                        po[:, 0, 0:HB], lhsT, v_sb[:, kb, 0:HB],
                        start=(kb == 0), stop=(kb == kb_hi),
                    )
                    nc.tensor.matmul(
                        po[:, 1, 0 : HB + 1], lhsT, v_sb[:, kb, HB : D + 1],
                        start=(kb == 0), stop=(kb == kb_hi),
                    )
                rec = small.tile([P, 1], F32, tag="rec")
                nc.vector.reciprocal(rec, po[:, 1, HB : HB + 1])
                if qb == NQB - 1:
                    # final block: normalize + store in quarters pipelined
                    # across both DMA queues to shorten the kernel tail.
                    for qtr in range(4):
                        b = (qtr % 2) * P
                        nc.vector.tensor_scalar_mul(
                            og[:, j, qtr * P : (qtr + 1) * P],
                            po[:, qtr // 2, b : b + P], rec,
                        )
                        dq = nc.sync if qtr % 2 == 0 else nc.scalar
                        dq.dma_start(
                            out=out_d[qb * P : (qb + 1) * P, qtr * P : (qtr + 1) * P],
                            in_=og[:, j, qtr * P : (qtr + 1) * P],
                        )
                else:
                    nc.vector.tensor_scalar_mul(og[:, j, 0:HB], po[:, 0, 0:HB], rec)
                    nc.vector.tensor_scalar_mul(og[:, j, HB:D], po[:, 1, 0:HB], rec)
                    nc.sync.dma_start(
                        out=out_d[qb * P : (qb + 1) * P, :], in_=og[:, j, :]
                    )

    nc.compile()
    return nc


def _get_nc(causal=True):
    key = ("nc", causal)
    if key not in _CACHE:
        _CACHE[key] = _build(causal)
    return _CACHE[key]


def _make_in_maps(q, k, v, Wq, bq, Wk, Wv, bv):
    import ml_dtypes

    mdt = ml_dtypes.bfloat16
    q = np.asarray(q, dtype=np.float32)
    k = np.asarray(k, dtype=np.float32)
    v = np.asarray(v, dtype=np.float32)

    def xq(x):  # [s, d] -> [p, sc, dc, qw] with d = dc*P + p, s = sc*QW + qw
        xt = np.ascontiguousarray(x.T).reshape(DC, P, NQC, QW)
        return np.ascontiguousarray(xt.transpose(1, 2, 0, 3)).astype(mdt)

    def xkv(x):  # [s, d] -> [p, sb, dc, ss] with d = dc*P + p, s = sb*P + ss
        xt = np.ascontiguousarray(x.T).reshape(DC, P, NQB, P)
        return np.ascontiguousarray(xt.transpose(1, 2, 0, 3)).astype(mdt)

    # host-precomputed weight products: N = Wq^T Wk, u = Wk^T bq.
    # nt layout [p, dcm, dpc, 0:128] = N[dpc*P+p, dcm*P+m]; u at col 128
    # of each dcm's dpc=0 plane (bias for the ht store).
    NT = np.asarray(Wq, np.float32).T @ np.asarray(Wk, np.float32)  # [d1, d2]
    u = np.asarray(Wk, np.float32).T @ np.asarray(bq, np.float32)  # [d]
    nt_t = np.zeros((P, DC, D + 2), np.float32)
    nt_t[:, :, :D] = NT.reshape(DC, P, D).transpose(1, 0, 2)
    nt_t[:, :, D] = u.reshape(DC, P).transpose(1, 0)
    nt_t = np.ascontiguousarray(nt_t).astype(mdt)
    wt = np.asarray(Wv, np.float32).T.reshape(DC, P, D)
    wv_t = np.ascontiguousarray(wt.transpose(1, 0, 2)).astype(mdt)
    bvb = np.ascontiguousarray(
        np.tile(np.asarray(bv, dtype=np.float32)[None, :], (P, 1))
    ).astype(mdt)
    cm = np.triu(np.ones((P, P), dtype=np.float32)).astype(mdt)  # cm[kk,qq]=qq>=kk
    in_maps = []
    for c in range(N_CORES):
        in_maps.append(
            {
                "qT": xq(q[c]),
                "kT": xkv(k[c]),
                "vT": xkv(v[c]),
                "ntT": nt_t,
                "wvT": wv_t,
                "bvb": bvb,
                "cm": cm,
            }
        )
    return in_maps


def _run(in_maps, trace=False, causal=True):
    from concourse.bass_utils import run_bass_kernel_spmd

    nc = _get_nc(causal)
    res = run_bass_kernel_spmd(
        nc, in_maps, core_ids=list(range(N_CORES)), trace=trace
    )
    out = np.stack(
        [np.asarray(res.results[c]["out"]).astype(np.float32) for c in range(N_CORES)],
        axis=0,
    )
    return out, res


def _mask_is_causal(mask):
    m = np.asarray(mask).reshape(S, S).astype(bool)
    if m.all():
        return False  # attend-to-everything mask: run the dense variant
    tril = np.tril(np.ones((S, S), dtype=bool))
    if np.array_equal(m, tril):
        return True
    raise ValueError("unsupported mask pattern (expected causal or all-ones)")


def kernel(q, k, v, mask, Wq, bq, Wk, bk, Wv, bv):
    q = np.asarray(q, dtype=np.float32)
    assert q.shape == (B, S, D), f"unexpected q shape {q.shape}"
    causal = _mask_is_causal(mask)
    in_maps = _make_in_maps(q, k, v, Wq, bq, Wk, Wv, bv)
    out, _ = _run(in_maps, trace=False, causal=causal)
    return out
